# revision 1
# baseline (speedup 1.0000x reference)
"""Graphormer layer (LocalSubgraphEncoder) Trainium2 Bass kernel.

Sharding: node-parallel over 8 cores. Core i computes the FULL layer output
for query nodes [512*i, 512*i+512): all 8 heads of attention over all 4096
key nodes, edge-type bias, softmax, output projection, residual, LayerNorm.
No cross-core communication; host concatenates row slices.

Score layout is transposed (S^T: dst/key nodes on partitions, query nodes on
the free axis) so the softmax denominator comes from an appended ones-column
on V (one matmul yields numerator and denominator). Edge bias is applied
multiplicatively after exp: P = exp(S)*(1+D), D = expm1(bias) scattered
densely by GPSIMD local_scatter with per-partition indices.
"""
import os
import sys
import math
import numpy as np

sys.path.insert(0, "/opt/trn_rl_repo")
import ml_dtypes  # noqa: E402
from concourse import bacc, bass, mybir, tile  # noqa: E402
from concourse.bass_utils import run_bass_kernel_spmd  # noqa: E402

N, D, H, E, NT = 4096, 256, 8, 131072, 16
DH = D // H            # 32
NCORES = 8
Q = N // NCORES        # 512 query nodes per core
KB = 128               # dst-node block (partition dim)
NKB = N // KB          # 32
NPAIR = NKB // 2       # 16 (two k-blocks share one [128,1024] score tile)
LN_EPS = 1e-5
SCALE = 1.0 / math.sqrt(DH)

f32 = mybir.dt.float32
f32r = mybir.dt.float32r
bf16 = mybir.dt.bfloat16
i16 = mybir.dt.int16
EXP = mybir.ActivationFunctionType.Exp
IDENT = mybir.ActivationFunctionType.Identity
SQRT = mybir.ActivationFunctionType.Sqrt
ADD = mybir.AluOpType.add
MULT = mybir.AluOpType.mult

USE_F32R = False
QK_BF16 = os.environ.get("QK_BF16", "1") == "1"

_PROG_CACHE = {}
LAST_RESULTS = None


def mm_dt(ap):
    return ap


def build_program(NI):
    nc = bacc.Bacc(
        "TRN2", target_bir_lowering=False, debug=False, num_devices=NCORES
    )

    def din(name, shape, dt):
        return nc.dram_tensor(name, shape, dt, kind="ExternalInput").ap()

    xT = din("xT", [D, N], f32)
    posT = din("posT", [D, N], f32)
    xqT = din("xqT", [D, Q], f32)
    posqT = din("posqT", [D, Q], f32)
    Wq_d = din("Wq", [D, D], f32)
    Wk_d = din("Wk", [D, D], f32)
    Wv_d = din("Wv", [D, D], f32)
    Wo_d = din("Wo", [D, D], f32)
    bq_d = din("bq", [D, 1], f32)
    bk_d = din("bk", [D, 1], f32)
    bv_d = din("bv", [1, D], f32)
    bo_d = din("bo", [D, 1], f32)
    gam_d = din("gamma", [D, 1], f32)
    bet_d = din("beta", [D, 1], f32)
    SC = H * NPAIR * NI
    sidx_d = din("sidx", [KB, SC], i16)
    sval_d = din("sval", [KB, SC], bf16)
    outT = nc.dram_tensor("outT", [D, Q], f32, kind="ExternalOutput").ap()

    with tile.TileContext(nc) as tc:
        from contextlib import ExitStack

        with ExitStack() as ctx:
            cpool = ctx.enter_context(tc.tile_pool(name="consts", bufs=1))

            def ctile(shape, dt, tag):
                return cpool.tile(shape, dt, tag=tag, name=tag)

            # ---- persistent SBUF residents ----
            hT = [ctile([KB, N], f32, f"hT{c}") for c in range(2)]
            qk_dt = bf16 if QK_BF16 else f32
            kT = [ctile([KB, N], qk_dt, f"kT{c}") for c in range(2)]
            qT = [ctile([KB, Q], f32, f"qT{c}") for c in range(2)]
            qTb = [ctile([KB, Q], qk_dt, f"qTb{c}") for c in range(2)]
            xq = [ctile([KB, Q], f32, f"xq{c}") for c in range(2)]
            attn = [ctile([KB, Q], f32, f"attn{c}") for c in range(2)]
            wq = [ctile([KB, D], f32, f"wq{c}") for c in range(2)]
            wk = [ctile([KB, D], f32, f"wk{c}") for c in range(2)]
            wv = [ctile([KB, D], f32, f"wv{c}") for c in range(2)]
            wo = [ctile([KB, D], f32, f"wo{c}") for c in range(2)]
            bq = [ctile([KB, 1], f32, f"bq{c}") for c in range(2)]
            bk = [ctile([KB, 1], f32, f"bk{c}") for c in range(2)]
            bo = [ctile([KB, 1], f32, f"bo{c}") for c in range(2)]
            gam = [ctile([KB, 1], f32, f"gam{c}") for c in range(2)]
            bet = [ctile([KB, 1], f32, f"bet{c}") for c in range(2)]
            bv_r = ctile([1, D], f32, "bv_r")
            ones_1x128 = ctile([1, KB], f32, "o1x128")
            ones_1x32 = ctile([1, 32], f32, "o1x32")
            ones_128x1 = ctile([KB, 1], f32, "o128x1")
            vw = ctile([KB, NKB, H, DH + 1], bf16, "vw")  # V with ones col
            sidx = ctile([KB, SC], i16, "sidx")
            sval = ctile([KB, SC], bf16, "sval")

            # ---- loads ----
            for c in range(2):
                sl = slice(c * KB, (c + 1) * KB)
                nc.sync.dma_start(out=hT[c][:], in_=xT[sl, :])
                nc.sync.dma_start(out=xq[c][:], in_=xqT[sl, :])
                nc.sync.dma_start(out=wq[c][:], in_=Wq_d[sl, :])
                nc.sync.dma_start(out=wk[c][:], in_=Wk_d[sl, :])
                nc.sync.dma_start(out=wv[c][:], in_=Wv_d[sl, :])
                nc.sync.dma_start(out=wo[c][:], in_=Wo_d[sl, :])
                nc.sync.dma_start(out=bq[c][:], in_=bq_d[sl, :])
                nc.sync.dma_start(out=bk[c][:], in_=bk_d[sl, :])
                nc.sync.dma_start(out=bo[c][:], in_=bo_d[sl, :])
                nc.sync.dma_start(out=gam[c][:], in_=gam_d[sl, :])
                nc.sync.dma_start(out=bet[c][:], in_=bet_d[sl, :])
            nc.sync.dma_start(out=bv_r[:], in_=bv_d[:])
            nc.sync.dma_start(out=sidx[:], in_=sidx_d[:])
            nc.sync.dma_start(out=sval[:], in_=sval_d[:])
            nc.vector.memset(ones_1x128[:], 1.0)
            nc.vector.memset(ones_1x32[:], 1.0)
            nc.vector.memset(ones_128x1[:], 1.0)

            # h = x + pos (transposed layout), done in-place on hT tiles
            tmp_pool = ctx.enter_context(tc.tile_pool(name="tmp", bufs=2))
            for c in range(2):
                sl = slice(c * KB, (c + 1) * KB)
                pt = tmp_pool.tile([KB, N], f32, tag="post", name="post")
                nc.sync.dma_start(out=pt[:], in_=posT[sl, :])
                nc.vector.tensor_add(hT[c][:], hT[c][:], pt[:])
                pq = tmp_pool.tile([KB, Q], f32, tag="posq", name="posq")
                nc.sync.dma_start(out=pq[:], in_=posqT[sl, :])
                nc.vector.tensor_add(qT[c][:], xq[c][:], pq[:])  # qT holds hq for now

            # ---- projections ----
            with tc.tile_pool(name="pps", bufs=3, space="PSUM") as pps:
                # K^T [256, 4096] and Q^T [256, 512] (head-major partitions)
                for c in range(2):  # output half (which 128 out-dims)
                    for nb in range(8):  # node blocks of 512
                        ps = pps.tile([KB, 512], f32, tag="proj", name="proj")
                        for kc in range(2):
                            nc.tensor.matmul(
                                ps[:],
                                (wk[kc][:, c * KB:(c + 1) * KB]),
                                (hT[kc][:, nb * 512:(nb + 1) * 512]),
                                start=(kc == 0),
                                stop=(kc == 1),
                            )
                        nc.scalar.activation(
                            kT[c][:, nb * 512:(nb + 1) * 512], ps[:],
                            IDENT, bias=bk[c][:], scale=1.0,
                        )
                for c in range(2):
                    ps = pps.tile([KB, 512], f32, tag="proj", name="proj")
                    hq_c = qT  # qT currently holds hq
                    for kc in range(2):
                        nc.tensor.matmul(
                            ps[:],
                            (wq[kc][:, c * KB:(c + 1) * KB]),
                            (hq_c[kc][:]),
                            start=(kc == 0),
                            stop=(kc == 1),
                        )
                    nc.scalar.activation(
                        qTb[c][:], ps[:], IDENT, bias=bq[c][:], scale=1.0
                    )
                # V (node-major) + bias + ones column -> vw
                nc.vector.memset(vw[:, :, :, DH], 1.0)
                for nb in range(NKB):
                    psv = pps.tile([KB, H, DH], f32, tag="projv", name="projv")
                    for kc in range(2):
                        nc.tensor.matmul(
                            psv[:, :, :],
                            (hT[kc][:, nb * KB:(nb + 1) * KB]),
                            (wv[kc][:]),
                            start=(kc == 0),
                            stop=False,
                        )
                    nc.tensor.matmul(
                        psv[:, :, :], ones_1x128[:], bv_r[:],
                        start=False, stop=True,
                    )
                    nc.vector.tensor_copy(vw[:, nb, :, 0:DH], psv[:, :, :])

            # ---- attention ----
            with ExitStack() as actx:
                sps = actx.enter_context(
                    tc.tile_pool(name="sps", bufs=2, space="PSUM")
                )
                ops = actx.enter_context(
                    tc.tile_pool(name="ops", bufs=2, space="PSUM")
                )
                mps = actx.enter_context(
                    tc.tile_pool(name="mps", bufs=2, space="PSUM")
                )
                spool = actx.enter_context(tc.tile_pool(name="spool", bufs=3))
                zpool = actx.enter_context(tc.tile_pool(name="zpool", bufs=2))

                for h in range(H):
                    mh, pbase = h // 4, 32 * (h % 4)
                    psl = slice(pbase, pbase + 32)
                    oacc = ops.tile([DH + 1, Q], f32, tag="oacc", name="oacc")
                    for t in range(NPAIR):
                        sp = sps.tile([KB, 2 * Q], f32, tag="sT", name="sT")
                        for j in range(2):
                            kb = 2 * t + j
                            nc.tensor.matmul(
                                sp[:, j * Q:(j + 1) * Q],
                                kT[mh][psl, kb * KB:(kb + 1) * KB],
                                qTb[mh][psl, :],
                                start=True, stop=True,
                                tile_position=(pbase, 0),
                            )
                        p0 = spool.tile([KB, 2 * Q], bf16, tag="p0", name="p0")
                        nc.scalar.activation(p0[:], sp[:], EXP, scale=SCALE)
                        dd = spool.tile([KB, 2 * Q], bf16, tag="dd", name="dd")
                        off = (h * NPAIR + t) * NI
                        nc.gpsimd.local_scatter(
                            dd[:], sval[:, off:off + NI], sidx[:, off:off + NI],
                            channels=KB, num_elems=2 * Q, num_idxs=NI,
                        )
                        pf = spool.tile([KB, 2 * Q], bf16, tag="pf", name="pf")
                        nc.vector.scalar_tensor_tensor(
                            pf[:], dd[:], 1.0, p0[:], op0=ADD, op1=MULT
                        )
                        for j in range(2):
                            kb = 2 * t + j
                            nc.tensor.matmul(
                                oacc[:],
                                vw[:, kb, h, :],
                                pf[:, j * Q:(j + 1) * Q],
                                start=(t == 0 and j == 0),
                                stop=(t == NPAIR - 1 and j == 1),
                                skip_group_check=True,
                            )
                    # normalize: attn_h = oacc[0:32] / oacc[32]
                    zinv = zpool.tile([1, Q], f32, tag="zinv", name="zinv")
                    nc.vector.reciprocal(zinv[:], oacc[DH:DH + 1, :])
                    zb = mps.tile([32, Q], f32, tag="zb", name="zb")
                    nc.tensor.matmul(
                        zb[:], ones_1x32[:], zinv[:], start=True, stop=True
                    )
                    zsb = zpool.tile([32, Q], f32, tag="zsb", name="zsb")
                    nc.scalar.copy(zsb[:], zb[:])
                    nc.vector.tensor_mul(
                        attn[mh][psl, :], oacc[0:DH, :], zsb[:]
                    )

            # ---- output projection + residual + LayerNorm ----
            with ExitStack() as ectx:
                eps_ps = ectx.enter_context(
                    tc.tile_pool(name="eps", bufs=1, space="PSUM")
                )
                rps = ectx.enter_context(
                    tc.tile_pool(name="rps", bufs=2, space="PSUM")
                )
                epool = ectx.enter_context(tc.tile_pool(name="epool", bufs=2))
                out2 = [epool.tile([KB, Q], f32, tag=f"out2_{c}", name=f"out2_{c}")
                        for c in range(2)]
                for c in range(2):
                    op_ps = rps.tile([KB, Q], f32, tag="oproj", name="oproj")
                    for kc in range(2):
                        nc.tensor.matmul(
                            op_ps[:],
                            (wo[kc][:, c * KB:(c + 1) * KB]),
                            (attn[kc][:]),
                            start=(kc == 0),
                            stop=(kc == 1),
                        )
                    # out2 = (psum + bo) + x
                    nc.vector.scalar_tensor_tensor(
                        out2[c][:], op_ps[:], bo[c][:], xq[c][:],
                        op0=ADD, op1=ADD,
                    )
                mu_ps = eps_ps.tile([1, Q], f32, tag="mu", name="mu")
                for c in range(2):
                    nc.tensor.matmul(
                        mu_ps[:], ones_128x1[:], out2[c][:],
                        start=(c == 0), stop=(c == 1),
                    )
                s2_ps = eps_ps.tile([1, Q], f32, tag="s2", name="s2")
                for c in range(2):
                    sq = epool.tile([KB, Q], f32, tag="sq", name="sq")
                    nc.scalar.square(sq[:], out2[c][:])
                    nc.tensor.matmul(
                        s2_ps[:], ones_128x1[:], sq[:],
                        start=(c == 0), stop=(c == 1),
                        skip_group_check=True,
                    )
                mu = epool.tile([1, Q], f32, tag="mu_s", name="mu_s")
                nc.vector.tensor_scalar_mul(mu[:], mu_ps[:], 1.0 / D)
                m2 = epool.tile([1, Q], f32, tag="m2", name="m2")
                nc.vector.tensor_mul(m2[:], mu[:], mu[:])
                var = epool.tile([1, Q], f32, tag="var", name="var")
                nc.vector.scalar_tensor_tensor(
                    var[:], s2_ps[:], 1.0 / D, m2[:],
                    op0=MULT, op1=mybir.AluOpType.subtract,
                )
                sd = epool.tile([1, Q], f32, tag="sd", name="sd")
                epsT = epool.tile([1, 1], f32, tag="epsT", name="epsT")
                nc.vector.memset(epsT[:], LN_EPS)
                nc.scalar.activation(sd[:], var[:], SQRT, bias=epsT[:])
                rstd = epool.tile([1, Q], f32, tag="rstd", name="rstd")
                nc.vector.reciprocal(rstd[:], sd[:])
                mu_b = eps_ps.tile([KB, Q], f32, tag="mu_b", name="mu_b")
                nc.tensor.matmul(mu_b[:], ones_1x128[:], mu[:], start=True, stop=True)
                rstd_b = eps_ps.tile([KB, Q], f32, tag="rstd_b", name="rstd_b")
                nc.tensor.matmul(rstd_b[:], ones_1x128[:], rstd[:], start=True, stop=True)
                for c in range(2):
                    t1 = epool.tile([KB, Q], f32, tag="t1", name="t1")
                    nc.vector.tensor_sub(t1[:], out2[c][:], mu_b[:])
                    t2 = epool.tile([KB, Q], f32, tag="t2", name="t2")
                    nc.vector.scalar_tensor_tensor(
                        t2[:], t1[:], gam[c][:], rstd_b[:],
                        op0=MULT, op1=MULT,
                    )
                    o4 = epool.tile([KB, Q], f32, tag="o4", name="o4")
                    nc.scalar.activation(o4[:], t2[:], IDENT, bias=bet[c][:])
                    nc.sync.dma_start(
                        out=outT[c * KB:(c + 1) * KB, :], in_=o4[:]
                    )

    nc.compile()
    return nc


def _prep_edges(src, dst, bias_eh):
    """Dedupe (src,dst), sum biases; returns per-core local_scatter arrays."""
    key = src.astype(np.int64) * N + dst.astype(np.int64)
    uk, inv = np.unique(key, return_inverse=True)
    acc = np.zeros((len(uk), H), np.float32)
    np.add.at(acc, inv, bias_eh)
    usrc = (uk // N).astype(np.int32)
    udst = (uk % N).astype(np.int32)
    vals = np.expm1(acc)  # [U, H]

    per_core = []
    NI = 2
    for i in range(NCORES):
        sel = (usrc >> 9) == i
        s, d, v = usrc[sel], udst[sel], vals[sel]
        qloc = s & (Q - 1)
        kb = d >> 7
        t = kb >> 1
        col = ((kb & 1) << 9) + qloc
        p = d & (KB - 1)
        g = t * KB + p
        order = np.argsort(g, kind="stable")
        gs = g[order]
        counts = np.bincount(gs, minlength=NPAIR * KB)
        ni = int(counts.max()) if len(gs) else 1
        NI = max(NI, ni)
        per_core.append((t[order], p[order], col[order], v[order], counts))
    NI = (NI + 1) & ~1  # even
    cores_out = []
    for t_o, p_o, col_o, v_o, counts in per_core:
        slot = np.arange(len(t_o)) - np.repeat(
            np.cumsum(counts) - counts, counts
        )
        idx_arr = np.full((NPAIR, KB, NI), -1, np.int16)
        idx_arr[t_o, p_o, slot] = col_o.astype(np.int16)
        val_arr = np.zeros((H, NPAIR, KB, NI), np.float32)
        val_arr[:, t_o, p_o, slot] = v_o.T
        # pack: [KB, H*NPAIR*NI]
        sidx = np.broadcast_to(
            idx_arr.transpose(1, 0, 2)[:, None, :, :], (KB, H, NPAIR, NI)
        ).reshape(KB, H * NPAIR * NI).copy()
        sval = (
            val_arr.transpose(2, 0, 1, 3)
            .reshape(KB, H * NPAIR * NI)
            .astype(ml_dtypes.bfloat16)
        )
        cores_out.append((sidx, sval))
    return NI, cores_out


def kernel(**inputs):
    global LAST_RESULTS
    x = np.asarray(inputs["x"], np.float32)
    pos = np.asarray(inputs["pos_encoding"], np.float32)
    ei = np.asarray(inputs["edge_index"])
    et = np.asarray(inputs["edge_types"])
    emb = np.asarray(inputs["edge_emb"], np.float32)
    W = {k: np.ascontiguousarray(np.asarray(inputs[k], np.float32))
         for k in ("Wq", "Wk", "Wv", "Wo")}
    b = {k: np.asarray(inputs[k], np.float32).reshape(-1)
         for k in ("bq", "bk", "bv", "bo", "gamma", "beta")}

    bias_eh = emb[et]  # [E, H]
    NI, scat = _prep_edges(ei[0], ei[1], bias_eh)

    if NI not in _PROG_CACHE:
        _PROG_CACHE[NI] = build_program(NI)
    nc = _PROG_CACHE[NI]

    xT = np.ascontiguousarray(x.T)
    posT = np.ascontiguousarray(pos.T)
    col = lambda a: np.ascontiguousarray(a.reshape(D, 1))
    in_maps = []
    for i in range(NCORES):
        sl = slice(i * Q, (i + 1) * Q)
        sidx, sval = scat[i]
        in_maps.append({
            "xT": xT, "posT": posT,
            "xqT": np.ascontiguousarray(xT[:, sl]),
            "posqT": np.ascontiguousarray(posT[:, sl]),
            "Wq": W["Wq"], "Wk": W["Wk"], "Wv": W["Wv"], "Wo": W["Wo"],
            "bq": col(b["bq"]), "bk": col(b["bk"]),
            "bv": np.ascontiguousarray(b["bv"].reshape(1, D)),
            "bo": col(b["bo"]), "gamma": col(b["gamma"]),
            "beta": col(b["beta"]),
            "sidx": sidx, "sval": sval,
        })

    trace = os.environ.get("BASS_KERNEL_TRACE", "0") == "1"
    try:
        res = run_bass_kernel_spmd(
            nc, in_maps, list(range(NCORES)), trace=trace
        )
    except Exception:
        if not trace:
            raise
        res = run_bass_kernel_spmd(nc, in_maps, list(range(NCORES)))
    LAST_RESULTS = res

    out = np.empty((N, D), np.float32)
    for i in range(NCORES):
        out[i * Q:(i + 1) * Q, :] = np.asarray(
            res.results[i]["outT"], np.float32
        ).T
    return out



# revision 14
# speedup vs baseline: 1.4018x; 1.4018x over previous
"""Graphormer layer (LocalSubgraphEncoder) Trainium2 Bass kernel, v2.

Sharding: node-parallel over 8 cores. Core i computes the full layer output
for query nodes [512*i, 512*i+512): all 8 heads of attention over all 4096
key nodes, edge-type bias, softmax, output projection, residual, LayerNorm.
No cross-core communication; host concatenates row slices.

v2 design (from perfetto analysis of v1: PE saturated by unpacked K=32
matmuls, GPSIMD dense local_scatter, STT stuck in 1x mode):
 - all matmuls bf16; 2-head row-packing for QK (tile_position row groups)
   and 2-head column-packing for PV / denominator matmuls.
 - scores layout S^T [keys(part), queries(free)]: softmax denominator z
   comes from a packed ones-vector matmul into a shared PSUM bank.
 - edge bias applied multiplicatively AFTER exp: P = exp(S) * F where
   F = exp(scattered bias) is precomputed DENSE on the host and streamed
   from HBM (33.5 MB/core) -> one 2x-mode DVE tensor_tensor per tile;
   GPSIMD does nothing.
 - ACT (ScalarE) does exclusively the exp drain PSUM->SBUF bf16 in
   [128,1024] tiles: the ~128 us floor every design shares.
 - biases fused into DVE copies (per-partition scalar AP) or rank-1 PE
   matmuls; LayerNorm scale/shift via outer-product matmuls.
"""
import os
import sys
import math
import numpy as np

sys.path.insert(0, "/opt/trn_rl_repo")
import ml_dtypes  # noqa: E402
from concourse import bacc, bass, mybir, tile  # noqa: E402
from concourse.bass_utils import run_bass_kernel_spmd  # noqa: E402

N, D, H, E, NT = 4096, 256, 8, 131072, 16
DH = D // H            # 32
NCORES = 8
Q = N // NCORES        # 512 query nodes per core
KB = 128               # key-node block (partition dim)
NKB = N // KB          # 32
NPAIR = NKB // 2       # 16 (two key-blocks per [128,1024] score tile)
LN_EPS = 1e-5
SCALE = 1.0 / math.sqrt(DH)

f32 = mybir.dt.float32
bf16 = mybir.dt.bfloat16
EXP = mybir.ActivationFunctionType.Exp
SQRT = mybir.ActivationFunctionType.Sqrt
ADD = mybir.AluOpType.add
MULT = mybir.AluOpType.mult
SUB = mybir.AluOpType.subtract

_PROG = None
LAST_RESULTS = None


def build_program():
    nc = bacc.Bacc(
        "TRN2", target_bir_lowering=False, debug=False, num_devices=NCORES
    )

    def din(name, shape, dt):
        return nc.dram_tensor(name, shape, dt, kind="ExternalInput").ap()

    hT_d = din("hT", [D, N], bf16)          # (x + pos)^T
    xqT_d = din("xqT", [D, Q], f32)         # x^T core slice (residual)
    Wq_d = din("Wq", [D, D], bf16)
    Wk_d = din("Wk", [D, D], bf16)
    Wv_d = din("Wv", [D, D], bf16)
    Wo_d = din("Wo", [D, D], bf16)
    bq_d = din("bq", [D, 1], f32)
    bk_d = din("bk", [D, 1], f32)
    bo_d = din("bo", [D, 1], f32)
    bv_d = din("bv", [1, D], bf16)
    gm_d = din("gm", [1, D], bf16)          # gamma row
    bt_d = din("bt", [1, D], bf16)          # beta row
    e128_d = din("e128", [KB, KB], bf16)    # block-broadcast matrix
    F_d = din("F", [2 * NPAIR * 4 * KB, 2 * Q], bf16)  # dense exp(bias)
    outT = nc.dram_tensor("outT", [D, Q], f32, kind="ExternalOutput").ap()

    hqT_d = din("hqT", [D, Q], bf16)        # h^T core query slice

    with tile.TileContext(nc) as tc:
        from contextlib import ExitStack

        with ExitStack() as ctx:
            cpool = ctx.enter_context(tc.tile_pool(name="consts", bufs=1))

            def ctile(shape, dt, tag):
                return cpool.tile(shape, dt, tag=tag, name=tag)

            # persistent SBUF residents
            hT = [ctile([KB, N], bf16, f"hT{c}") for c in range(2)]
            hq = [ctile([KB, Q], bf16, f"hq{c}") for c in range(2)]
            xq = [ctile([KB, Q], f32, f"xq{c}") for c in range(2)]
            wq = [ctile([KB, D], bf16, f"wq{c}") for c in range(2)]
            wk = [ctile([KB, D], bf16, f"wk{c}") for c in range(2)]
            wv = [ctile([KB, D], bf16, f"wv{c}") for c in range(2)]
            wo = [ctile([KB, D], bf16, f"wo{c}") for c in range(2)]
            bq = [ctile([KB, 1], f32, f"bq{c}") for c in range(2)]
            bk = [ctile([KB, 1], f32, f"bk{c}") for c in range(2)]
            bo = [ctile([KB, 1], f32, f"bo{c}") for c in range(2)]
            bv_r = ctile([1, D], bf16, "bv_r")
            gm = ctile([1, D], bf16, "gm")
            bt = ctile([1, D], bf16, "bt")
            e128 = ctile([KB, KB], bf16, "e128")
            kT = [ctile([KB, N], bf16, f"kT{c}") for c in range(2)]
            qTb = [ctile([KB, Q], bf16, f"qTb{c}") for c in range(2)]
            vSB = ctile([KB, NKB, D], bf16, "vSB")  # [key, kb, h*32+d]
            attnT = [ctile([KB, Q], bf16, f"attnT{c}") for c in range(2)]
            ones_1x128 = ctile([1, KB], bf16, "o1x128")
            ones_128x1 = ctile([KB, 1], bf16, "o128x1")
            ones_1xQ = ctile([1, Q], bf16, "o1xQ")
            epsT = ctile([1, 1], f32, "epsT")

            # ---- loads ----
            for c in range(2):
                sl = slice(c * KB, (c + 1) * KB)
                nc.sync.dma_start(out=hT[c][:], in_=hT_d[sl, :])
                nc.sync.dma_start(out=hq[c][:], in_=hqT_d[sl, :])
                nc.sync.dma_start(out=xq[c][:], in_=xqT_d[sl, :])
                nc.sync.dma_start(out=wq[c][:], in_=Wq_d[sl, :])
                nc.sync.dma_start(out=wk[c][:], in_=Wk_d[sl, :])
                nc.sync.dma_start(out=wv[c][:], in_=Wv_d[sl, :])
                nc.sync.dma_start(out=wo[c][:], in_=Wo_d[sl, :])
                nc.sync.dma_start(out=bq[c][:], in_=bq_d[sl, :])
                nc.sync.dma_start(out=bk[c][:], in_=bk_d[sl, :])
                nc.sync.dma_start(out=bo[c][:], in_=bo_d[sl, :])
            nc.sync.dma_start(out=bv_r[:], in_=bv_d[:])
            nc.sync.dma_start(out=gm[:], in_=gm_d[:])
            nc.sync.dma_start(out=bt[:], in_=bt_d[:])
            nc.sync.dma_start(out=e128[:], in_=e128_d[:])
            nc.vector.memset(ones_1x128[:], 1.0)
            nc.vector.memset(ones_128x1[:], 1.0)
            nc.vector.memset(ones_1xQ[:], 1.0)
            nc.vector.memset(epsT[:], LN_EPS)

            # ---- projections (all bf16, biases fused) ----
            with tc.tile_pool(name="pps", bufs=3, space="PSUM") as pps:
                # Q^T [2][128, 512] head-major partitions
                for mh in range(2):
                    ps = pps.tile([KB, Q], f32, tag="proj", name="proj")
                    for kc in range(2):
                        nc.tensor.matmul(
                            ps[:], wq[kc][:, mh * KB:(mh + 1) * KB], hq[kc][:],
                            start=(kc == 0), stop=(kc == 1),
                        )
                    nc.vector.tensor_scalar(
                        qTb[mh][:], ps[:], bq[mh][:], None, ADD
                    )
                # K^T [2][128, 4096]
                for mh in range(2):
                    for s in range(8):
                        ssl = slice(s * Q, (s + 1) * Q)
                        ps = pps.tile([KB, Q], f32, tag="proj", name="proj")
                        for kc in range(2):
                            nc.tensor.matmul(
                                ps[:], wk[kc][:, mh * KB:(mh + 1) * KB],
                                hT[kc][:, ssl],
                                start=(kc == 0), stop=(kc == 1),
                            )
                        nc.vector.tensor_scalar(
                            kT[mh][:, ssl], ps[:], bk[mh][:], None, ADD
                        )
                # V node-major [128, kb, 256] + bias via rank-1
                for kb_i in range(NKB):
                    ksl = slice(kb_i * KB, (kb_i + 1) * KB)
                    psv = pps.tile([KB, D], f32, tag="projv", name="projv")
                    for kc in range(2):
                        nc.tensor.matmul(
                            psv[:], hT[kc][:, ksl], wv[kc][:],
                            start=(kc == 0), stop=False,
                        )
                    nc.tensor.matmul(
                        psv[:], ones_1x128[:], bv_r[:],
                        start=False, stop=True,
                    )
                    nc.vector.tensor_copy(vSB[:, kb_i, :], psv[:])

            # ---- attention ----
            with ExitStack() as actx:
                sps = actx.enter_context(
                    tc.tile_pool(name="sps", bufs=3, space="PSUM")
                )
                ops = actx.enter_context(
                    tc.tile_pool(name="ops", bufs=1, space="PSUM")
                )
                zps = actx.enter_context(
                    tc.tile_pool(name="zps", bufs=1, space="PSUM")
                )
                spool = actx.enter_context(tc.tile_pool(name="spool", bufs=3))
                fpool = actx.enter_context(tc.tile_pool(name="fpool", bufs=8))
                npool = actx.enter_context(tc.tile_pool(name="npool", bufs=2))

                for mh in range(2):
                    oacc = ops.tile([KB, Q], f32, tag="oacc", name="oacc")
                    zt = zps.tile([KB, Q], f32, tag="zt", name="zt")
                    for t in range(NPAIR):
                        p0 = [None, None]
                        pf = [None, None]
                        ft = [None, None]
                        for pr in range(2):      # head pairs (2p, 2p+1)
                            # QK: 2-head row-packed, j-outer for packing runs
                            sg = [
                                sps.tile([KB, 2 * Q], f32, tag="sg", name="sg")
                                for _ in range(2)
                            ]
                            for j in range(2):
                                kb_i = 2 * t + j
                                ksl = slice(kb_i * KB, (kb_i + 1) * KB)
                                for hp in range(2):
                                    h4 = 2 * pr + hp
                                    psl = slice(32 * h4, 32 * h4 + 32)
                                    nc.tensor.matmul(
                                        sg[hp][:, j * Q:(j + 1) * Q],
                                        kT[mh][psl, ksl],
                                        qTb[mh][psl, :],
                                        start=True, stop=True,
                                        tile_position=(32 * h4, 0),
                                    )
                            for hp in range(2):
                                h4 = 2 * pr + hp
                                h = 4 * mh + h4
                                # exp (ACT) PSUM -> SBUF bf16
                                p0[hp] = spool.tile(
                                    [KB, 2 * Q], bf16, tag="p0", name="p0"
                                )
                                nc.scalar.activation(
                                    p0[hp][:], sg[hp][:], EXP, scale=SCALE
                                )
                                # F multiply (DVE 2x)
                                ft[hp] = fpool.tile(
                                    [KB, 2 * Q], bf16, tag="ft", name="ft"
                                )
                                row = ((mh * NPAIR + t) * 4 + h4) * KB
                                nc.sync.dma_start(
                                    out=ft[hp][:],
                                    in_=F_d[row:row + KB, :],
                                )
                                pf[hp] = spool.tile(
                                    [KB, 2 * Q], bf16, tag="pf", name="pf"
                                )
                                nc.vector.tensor_mul(
                                    pf[hp][:], p0[hp][:], ft[hp][:]
                                )
                            # PV: 2-head col-packed; then z
                            first = (t == 0)
                            last = (t == NPAIR - 1)
                            for j in range(2):
                                kb_i = 2 * t + j
                                for hp in range(2):
                                    h4 = 2 * pr + hp
                                    h = 4 * mh + h4
                                    nc.tensor.matmul(
                                        oacc[32 * h4:32 * h4 + 32, :],
                                        vSB[:, kb_i, 32 * h:32 * h + 32],
                                        pf[hp][:, j * Q:(j + 1) * Q],
                                        start=(first and j == 0),
                                        stop=(last and j == 1),
                                        tile_position=(0, 32 * h4),
                                        skip_group_check=True,
                                    )
                            for j in range(2):
                                for hp in range(2):
                                    h4 = 2 * pr + hp
                                    nc.tensor.matmul(
                                        zt[32 * h4:32 * h4 + 1, :],
                                        ones_128x1[:],
                                        pf[hp][:, j * Q:(j + 1) * Q],
                                        start=(first and j == 0),
                                        stop=(last and j == 1),
                                        tile_position=(0, 32 * h4),
                                        skip_group_check=True,
                                    )
                    # ---- normalize: attn = oacc * (1/z) broadcast ----
                    # z rows live at partitions {0,32,64,96}; batch recip on
                    # a [128, Q] tile (other partitions memset to 1.0).
                    zsb = npool.tile([KB, Q], f32, tag="zsb", name="zsb")
                    nc.vector.memset(zsb[:], 1.0)
                    for h4 in range(4):
                        nc.vector.tensor_copy(
                            zsb[32 * h4:32 * h4 + 1, :],
                            zt[32 * h4:32 * h4 + 1, :],
                        )
                    rz = npool.tile([KB, Q], f32, tag="rz", name="rz")
                    nc.vector.reciprocal_approx_fast(rz[:], zsb[:])
                    rzb = npool.tile([KB, Q], bf16, tag="rzb", name="rzb")
                    nc.vector.tensor_copy(rzb[:], rz[:])
                    zbp = sps.tile([KB, Q], f32, tag="sg", name="zbp")
                    nc.tensor.matmul(
                        zbp[:], e128[:], rzb[:], start=True, stop=True
                    )
                    zbs = npool.tile([KB, Q], f32, tag="zbs", name="zbs")
                    nc.vector.tensor_copy(zbs[:], zbp[:])
                    nc.vector.tensor_mul(attnT[mh][:], oacc[:], zbs[:])

            # ---- output projection + residual + LayerNorm ----
            with ExitStack() as ectx:
                rps = ectx.enter_context(
                    tc.tile_pool(name="rps", bufs=1, space="PSUM")
                )
                epool = ectx.enter_context(tc.tile_pool(name="epool", bufs=2))
                out2 = [
                    epool.tile([KB, Q], f32, tag=f"out2_{c}", name=f"out2_{c}")
                    for c in range(2)
                ]
                for c in range(2):
                    op_ps = rps.tile([KB, Q], f32, tag="oproj", name="oproj")
                    for mh in range(2):
                        nc.tensor.matmul(
                            op_ps[:],
                            wo[mh][:, c * KB:(c + 1) * KB],
                            attnT[mh][:],
                            start=(mh == 0), stop=(mh == 1),
                        )
                    # out2 = (psum + bo) + x
                    nc.vector.scalar_tensor_tensor(
                        out2[c][:], op_ps[:], bo[c][:], xq[c][:],
                        op0=ADD, op1=ADD,
                    )
                # stats: mu, s2 via ones matmuls (f32)
                ones_f = epool.tile([KB, 1], f32, tag="onesf", name="onesf")
                nc.vector.memset(ones_f[:], 1.0)
                mu_ps = rps.tile([1, Q], f32, tag="mu", name="mu")
                for c in range(2):
                    nc.tensor.matmul(
                        mu_ps[:], ones_f[:], out2[c][:],
                        start=(c == 0), stop=(c == 1),
                        skip_group_check=True,
                    )
                s2_ps = rps.tile([1, Q], f32, tag="s2", name="s2")
                for c in range(2):
                    sq = epool.tile([KB, Q], f32, tag="sq", name="sq")
                    nc.vector.tensor_mul(sq[:], out2[c][:], out2[c][:])
                    nc.tensor.matmul(
                        s2_ps[:], ones_f[:], sq[:],
                        start=(c == 0), stop=(c == 1),
                        skip_group_check=True,
                    )
                mu = epool.tile([1, Q], f32, tag="mu_s", name="mu_s")
                nc.vector.tensor_scalar_mul(mu[:], mu_ps[:], 1.0 / D)
                m2 = epool.tile([1, Q], f32, tag="m2", name="m2")
                nc.vector.tensor_mul(m2[:], mu[:], mu[:])
                var = epool.tile([1, Q], f32, tag="var", name="var")
                nc.vector.scalar_tensor_tensor(
                    var[:], s2_ps[:], 1.0 / D, m2[:], op0=MULT, op1=SUB,
                )
                sd = epool.tile([1, Q], f32, tag="sd", name="sd")
                nc.scalar.activation(sd[:], var[:], SQRT, bias=epsT[:])
                rstd = epool.tile([1, Q], f32, tag="rstd", name="rstd")
                nc.vector.reciprocal_approx_fast(rstd[:], sd[:])
                # broadcast tiles via outer products:
                # c1 = gamma (x) rstd ; c2 = beta (x) 1 - gamma (x) (mu*rstd)
                rstd_b = epool.tile([1, Q], bf16, tag="rstdb", name="rstdb")
                nc.vector.tensor_copy(rstd_b[:], rstd[:])
                mr = epool.tile([1, Q], f32, tag="mr", name="mr")
                nc.vector.tensor_mul(mr[:], mu[:], rstd[:])
                mrn = epool.tile([1, Q], bf16, tag="mrn", name="mrn")
                nc.vector.tensor_scalar_mul(mrn[:], mr[:], -1.0)
                for c in range(2):
                    csl = slice(c * KB, (c + 1) * KB)
                    c1p = rps.tile([KB, Q], f32, tag="c1", name="c1")
                    nc.tensor.matmul(
                        c1p[:], gm[:, csl], rstd_b[:], start=True, stop=True
                    )
                    # c2 = gamma (x) (-mu*rstd) + beta (x) 1
                    c2p = rps.tile([KB, Q], f32, tag="c2", name="c2")
                    nc.tensor.matmul(
                        c2p[:], gm[:, csl], mrn[:], start=True, stop=False
                    )
                    nc.tensor.matmul(
                        c2p[:], bt[:, csl], ones_1xQ[:],
                        start=False, stop=True,
                    )
                    t1 = epool.tile([KB, Q], f32, tag="t1", name="t1")
                    nc.vector.tensor_mul(t1[:], out2[c][:], c1p[:])
                    y = epool.tile([KB, Q], f32, tag="y", name="y")
                    nc.vector.tensor_add(y[:], t1[:], c2p[:])
                    nc.sync.dma_start(out=outT[csl, :], in_=y[:])

    nc.compile()
    return nc


def _prep_F(q_idx, k_idx, bias_eh):
    """Dense multiplicative bias F = exp(scattered bias), per core.

    Row-block order matches kernel consumption: [mh, t, h4, partition]."""
    key = q_idx.astype(np.int64) * N + k_idx.astype(np.int64)
    uk, inv = np.unique(key, return_inverse=True)
    acc = np.zeros((len(uk), H), np.float32)
    np.add.at(acc, inv, bias_eh)
    uq = (uk // N).astype(np.int32)
    ukey = (uk % N).astype(np.int32)
    vals16 = np.exp(acc).astype(ml_dtypes.bfloat16).view(np.uint16)

    Fs = []
    for i in range(NCORES):
        sel = (uq >> 9) == i
        q = uq[sel] & (Q - 1)
        k = ukey[sel]
        v = vals16[sel]
        t = k >> 8
        j = (k >> 7) & 1
        p = k & (KB - 1)
        col = j * Q + q
        F16 = np.full((2, NPAIR, 4, KB, 2 * Q), 0x3F80, np.uint16)
        for h in range(H):
            F16[h >> 2, t, h & 3, p, col] = v[:, h]
        Fs.append(
            np.ascontiguousarray(F16.reshape(2 * NPAIR * 4 * KB, 2 * Q))
            .view(ml_dtypes.bfloat16)
        )
    return Fs


def kernel(**inputs):
    global LAST_RESULTS, _PROG
    x = np.asarray(inputs["x"], np.float32)
    pos = np.asarray(inputs["pos_encoding"], np.float32)
    ei = np.asarray(inputs["edge_index"])
    et = np.asarray(inputs["edge_types"])
    emb = np.asarray(inputs["edge_emb"], np.float32)
    W = {k: np.asarray(inputs[k], np.float32) for k in ("Wq", "Wk", "Wv", "Wo")}
    b = {k: np.asarray(inputs[k], np.float32).reshape(-1)
         for k in ("bq", "bk", "bv", "bo", "gamma", "beta")}

    bias_eh = emb[et]  # [E, H]
    Fs = _prep_F(ei[0], ei[1], bias_eh)

    if _PROG is None:
        _PROG = build_program()
    nc = _PROG

    h = (x + pos).astype(np.float32)
    hT = np.ascontiguousarray(h.T.astype(ml_dtypes.bfloat16))
    xT = np.ascontiguousarray(x.T)
    Wb = {k: np.ascontiguousarray(w.astype(ml_dtypes.bfloat16))
          for k, w in W.items()}
    col = lambda a: np.ascontiguousarray(a.reshape(D, 1))
    row16 = lambda a: np.ascontiguousarray(
        a.reshape(1, D).astype(ml_dtypes.bfloat16)
    )
    e128 = np.zeros((KB, KB), np.float32)
    for h4 in range(4):
        e128[32 * h4, 32 * h4:32 * h4 + 32] = 1.0
    e128 = np.ascontiguousarray(e128.astype(ml_dtypes.bfloat16))

    in_maps = []
    for i in range(NCORES):
        sl = slice(i * Q, (i + 1) * Q)
        in_maps.append({
            "hT": hT,
            "hqT": np.ascontiguousarray(hT[:, sl]),
            "xqT": np.ascontiguousarray(xT[:, sl]),
            "Wq": Wb["Wq"], "Wk": Wb["Wk"], "Wv": Wb["Wv"], "Wo": Wb["Wo"],
            "bq": col(b["bq"]), "bk": col(b["bk"]), "bo": col(b["bo"]),
            "bv": row16(b["bv"]), "gm": row16(b["gamma"]),
            "bt": row16(b["beta"]), "e128": e128,
            "F": Fs[i],
        })

    trace = os.environ.get("BASS_KERNEL_TRACE", "0") == "1"
    try:
        res = run_bass_kernel_spmd(
            nc, in_maps, list(range(NCORES)), trace=trace
        )
    except Exception:
        if not trace:
            raise
        res = run_bass_kernel_spmd(nc, in_maps, list(range(NCORES)))
    LAST_RESULTS = res

    out = np.empty((N, D), np.float32)
    for i in range(NCORES):
        out[i * Q:(i + 1) * Q, :] = np.asarray(
            res.results[i]["outT"], np.float32
        ).T
    return out
